# revision 47
# baseline (speedup 1.0000x reference)
"""Trainium2 Bass kernel for nn_DilatedMask: 33x33 binary mask dilation.

Computes, for x of shape (8, 2048, 2048, 1) float32 with values {0,1}:
    mask = (x == 0)
    y    = sliding-window max of mask over a 33x33 window (SAME padding),
           as uint8.

Strategy (per NeuronCore, pure data parallel over the batch of 8):
  y = 1 iff the 33x33 window contains a zero of x.  Equivalently, with
  boxsum_h(x) the 33-tap column sum (clipped at borders) and count_h the
  per-row tap count:
      s1b = (boxsum_h(x) < count_h)        "column-window has a zero"
      y   = (boxsum_w(s1b) > 0)            "any such column in w-window"
  Both banded sums run on the TensorEngine as matmuls with the image tile
  as the *stationary* operand, which transposes each pass's output:
  pass 1 emits S1^T [w, h]; pass 2 lands back in natural [h, w].

  The input streams in via gpsimd SWDGE cast-DMAs (f32 -> fp8 during the
  DMA), so no engine ever touches the f32 data: the PE consumes x
  directly and the thresholds absorb the mask inversion.  PSUM
  evacuations (the compare ops) round-robin across DVE / ACT / GpSimd.

Pass-1 is banded over h_out (bands below); within a band, per w-tile all
contributing k-strips accumulate into one PSUM bank.  Pass-2 runs per
output row-strip with the w-tile loop outermost so each stationary loads
exactly once per strip.
"""

from contextlib import ExitStack

import numpy as np
import ml_dtypes

RADIUS = 16
SE = 2 * RADIUS + 1  # 33
P = 128
BANK = 512  # PSUM bank width in f32 elements
H = W = 2048
N_CORES = 8

# Pass-1 h_out bands: narrow at the start so the evac/pass-2 engines get
# work as soon as the first strips land; 512-wide in the middle to keep
# LDWEIGHTS duplication low; 256 at the end so the post-stream tail (work
# gated on the final strip) is short.
BANDS_H = [(0, 128), (128, 256), (256, 512), (512, 1024), (1024, 1536),
           (1536, 1792), (1792, 2048)]


def band_np() -> np.ndarray:
    """Band matrix chunk [128, 160]: band[p, j] = 1 iff j-32 <= p <= j."""
    p = np.arange(P)[:, None]
    j = np.arange(P + 2 * RADIUS)[None, :]
    return ((p <= j) & (p >= j - 2 * RADIUS)).astype(np.float32)


def cnt_np() -> np.ndarray:
    """Threshold counts, bf16 [128, 768]:
    cols 0..511   -> count_h(j) for h_out j in [0, 512)
    cols 512..767 -> count_h(1792 + j) for j in [0, 256)
    count_h(i) = min(i + 16, 2047) - max(i - 16, 0) + 1
    """
    i = np.arange(H)
    cnt = np.minimum(i + RADIUS, H - 1) - np.maximum(i - RADIUS, 0) + 1
    cols = np.concatenate([cnt[0:512], cnt[1792:2048]]).astype(np.float32)
    return np.broadcast_to(cols[None, :], (P, 768)).copy()


def _dedupe_ldweights(nc):
    """Remove back-to-back duplicate LDWEIGHTS in the PE stream.

    Tile lowers every matmul to LDWEIGHTS+MATMUL; consecutive matmuls that
    share a stationary reload identical weights.  The PE pairs each MATMUL
    with the most recent preceding LDWEIGHTS, so the reload is dead.
    Only drops loads with empty sync_info.
    """
    import concourse.mybir as mybir

    for blk in nc.m.functions[0].blocks:
        insts = list(blk.instructions)
        keep = []
        remap = {}
        last_sig = None
        for i in insts:
            if i.engine == mybir.EngineType.PE:
                tn = type(i).__name__
                if tn == "InstLdweights":
                    ap = i.ins[0]
                    sig = (ap.memref, ap.offset, str(ap.ap), str(ap.dtype),
                           bool(i.is_transpose), str(i.perf_mode))
                    si = i.sync_info
                    clean = si is None or (
                        not si.on_wait and not si.on_update
                    )
                    if sig == last_sig and clean:
                        remap[i.name] = last_name
                        continue
                    last_sig = sig
                    last_name = i.name
                elif tn != "InstMatmult":
                    last_sig = None
            keep.append(i)
        if remap:
            for i in keep:
                i.remap_dependency_names(remap)
            blk.instructions = keep


def build_program(h: int = H, w: int = W):
    """Build the per-core Bass program (SPMD, identical on all cores)."""
    import concourse.bass as bass
    import concourse.mybir as mybir
    import concourse.tile as tile
    from concourse import bacc

    f32 = mybir.dt.float32
    fp8 = mybir.dt.float8e4
    bf16 = mybir.dt.bfloat16
    u8 = mybir.dt.uint8

    nt_h = h // P
    nt_w = w // P
    bands = BANDS_H

    nc = bacc.Bacc("TRN2", target_bir_lowering=False, debug=False)
    # const AP for the ACT sign(33 - x) bias
    _t33 = nc.alloc_sbuf_tensor("const-f32-33", [P, 1], f32)
    nc.gpsimd.memset(_t33.ap(), 33.0)
    nc.const_aps.aps[(f32, 33.0)] = _t33.ap()
    x_ap = nc.dram_tensor("x", [h, w], f32, kind="ExternalInput").ap()
    band8_ap = nc.dram_tensor(
        "band8", [P, P + 2 * RADIUS], fp8, kind="ExternalInput"
    ).ap()
    cnt_ap = nc.dram_tensor("cnt", [P, 768], bf16, kind="ExternalInput").ap()
    y_ap = nc.dram_tensor("y", [h, w], u8, kind="ExternalOutput").ap()

    # Output row-strip groups per store DMA: pairs through the body, single
    # strips at the end so the final (fully exposed) store is small and the
    # second-to-last overlaps the last strip's compute.
    STORE_GROUPS = [(0, 2), (2, 4), (4, 6), (6, 8), (8, 10), (10, 12),
                    (12, 14), (14, 16)]

    with tile.TileContext(nc) as tc, ExitStack() as ctx:
        const_pool = ctx.enter_context(tc.tile_pool(name="const", bufs=1))
        m_pool = ctx.enter_context(tc.tile_pool(name="m", bufs=nt_h))
        s1_pool = ctx.enter_context(
            tc.tile_pool(name="s1", bufs=len(BANDS_H) * nt_w)
        )
        psA_pool = ctx.enter_context(tc.tile_pool(name="psA", bufs=4, space="PSUM"))
        psB_pool = ctx.enter_context(tc.tile_pool(name="psB", bufs=2, space="PSUM"))
        # One buffer per store group: a shared rotation would make late
        # evacs wait on the (deliberately delayed) early stores.
        out_pool = ctx.enter_context(
            tc.tile_pool(name="out", bufs=len(STORE_GROUPS))
        )

        # Input: 16 row strips, cast f32 -> fp8 {0,1} inside the SWDGE DMA.
        # All gens are emitted first so the gpsimd sequencer finishes
        # descriptor generation before any evac work lands on it.
        m_tiles = []
        for kt in range(nt_h):
            m = m_pool.tile([P, w], fp8, tag="m", name=f"m{kt}")
            nc.gpsimd.dma_start(out=m[:], in_=x_ap[kt * P : (kt + 1) * P, :])
            m_tiles.append(m)

        band8_t = const_pool.tile([P, P + 2 * RADIUS], fp8, tag="band8")
        nc.sync.dma_start(out=band8_t[:], in_=band8_ap[:, :])
        cnt_t = const_pool.tile([P, 768], bf16, tag="cnt")
        nc.sync.dma_start(out=cnt_t[:], in_=cnt_ap[:, :])

        # Persistent PSUM tiles (same memref -> PE WAW stays program-order).
        psA_tiles = [
            psA_pool.tile([P, BANK], f32, tag="psA", name=f"psA{i}")
            for i in range(4)
        ]
        psB_tiles = [
            psB_pool.tile([P, 2 * BANK], f32, tag="psB", name=f"psB{i}")
            for i in range(2)
        ]
        nA = [0]
        nB = [0]

        def next_psA():
            t = psA_tiles[nA[0] % len(psA_tiles)]
            nA[0] += 1
            return t

        def next_psB():
            t = psB_tiles[nB[0] % len(psB_tiles)]
            nB[0] += 1
            return t

        # s1 strips: S1^T [w', h] in fp8, one tile per (band, w-tile) so a
        # band's evac writes never alias an earlier band's pass-2 reads
        # (avoids false WAR serialization if dependency tracking is
        # tile-granular).
        s1_tiles = {
            (bi, wt): s1_pool.tile(
                [P, bhi - blo], fp8, tag="s1", name=f"s1_{bi}_{wt}"
            )
            for bi, (blo, bhi) in enumerate(bands)
            for wt in range(nt_w)
        }
        ht_band = {}
        for bi, (blo, bhi) in enumerate(bands):
            for t in range(blo // P, bhi // P):
                ht_band[t] = (bi, blo)
        yt_tiles = {}
        done_ht = {}

        # --- Evacuation engines -------------------------------------------
        # PSUM -> SBUF compares split across DVE and ACT (gpsimd has no
        # PSUM access on TRN2).  Greedy balance on estimated cost; compares
        # against the cnt tile need tensor_tensor and are DVE-only.
        ev_cost = {"v": 0.0, "a": 0.0}

        # Fixed costs include the ~2-3 Tile semaphore ops each evac puts on
        # the engine queue (~0.18us each), which showed up as real ACT/DVE
        # occupancy in traces.
        def pick_engine(cols, force_v=False):
            cv = ev_cost["v"] + 0.13 + cols * 1.07e-3
            ca = ev_cost["a"] + 0.25 + cols * 0.95e-3
            eng = "v" if (force_v or cv <= ca) else "a"
            ev_cost[eng] = cv if eng == "v" else ca
            return eng

        def evac_p1_uniform(dst_ap, src_ap, cols):
            eng = pick_engine(cols)
            if eng == "a":
                # sign(33 - boxsum): 1 iff boxsum < 33
                nc.scalar.activation(
                    dst_ap, src_ap, mybir.ActivationFunctionType.Sign,
                    bias=33.0, scale=-1.0,
                )
            else:
                nc.vector.tensor_scalar(dst_ap, src_ap, 33.0, None,
                                        mybir.AluOpType.is_lt)

        def evac_p1(dst_ap, psA, cnt_slice, blo, bhi):
            """s1b = (boxsum < count).  cnt_slice None -> uniform 33.

            Only the outermost RADIUS h_out columns have counts != 33; the
            cnt compare (DVE-only tensor_tensor) covers just those 16
            columns so the bulk stays eligible for either engine.
            """
            cols = bhi - blo
            if cnt_slice is None:
                evac_p1_uniform(dst_ap, psA[:, 0:cols], cols)
                return
            if blo < RADIUS:  # first band: edge cols at the front
                ev_cost["v"] += 0.16
                nc.vector.tensor_tensor(
                    dst_ap[:, 0:RADIUS], psA[:, 0:RADIUS],
                    cnt_slice[:, 0:RADIUS], mybir.AluOpType.is_lt,
                )
                evac_p1_uniform(
                    dst_ap[:, RADIUS:cols], psA[:, RADIUS:cols], cols - RADIUS
                )
            else:  # last band: edge cols at the back
                e = cols - RADIUS
                ev_cost["v"] += 0.16
                nc.vector.tensor_tensor(
                    dst_ap[:, e:cols], psA[:, e:cols],
                    cnt_slice[:, e:cols], mybir.AluOpType.is_lt,
                )
                evac_p1_uniform(dst_ap[:, 0:e], psA[:, 0:e], e)

        def evac_p2(dst_ap, psB, cols):
            """y = (boxsum_w > 0)."""
            eng = pick_engine(cols)
            src_ap = psB[:, 0:cols]
            if eng == "a":
                nc.scalar.sign(dst_ap, src_ap)
            else:
                nc.vector.tensor_scalar(dst_ap, src_ap, 0.5, None,
                                        mybir.AluOpType.is_gt)

        def cnt_slice_for(blo, bhi):
            """cnt tile slice for band [blo,bhi) or None if uniformly 33."""
            if blo < RADIUS:  # first band
                return cnt_t[:, blo : bhi]
            if bhi > h - RADIUS:  # last band
                off = 512 + (blo - 1792)
                return cnt_t[:, off : off + (bhi - blo)]
            return None

        # --- Pass 2 for one output row-strip ------------------------------
        # Matmul pieces must stay within one PSUM bank (512 f32); the
        # evac granularity is a psB half [128, 1024] (2 banks).
        HALF = 2 * BANK
        pieces_w = []  # (wt, lo, hi) over full w, split at BANK boundaries
        for wt in range(nt_w):
            win_lo = max(0, P * wt - RADIUS)
            win_hi = min(w, P * wt + P + RADIUS)
            lo = win_lo
            while lo < win_hi:
                hi = min(win_hi, (lo // BANK + 1) * BANK)
                pieces_w.append((wt, lo, hi))
                lo = hi
        q_last = {}
        bank_first = {}
        bank_last = {}
        for idx, (wt, lo, hi) in enumerate(pieces_w):
            q_last[lo // HALF] = idx
            b = lo // BANK
            if b not in bank_first:
                bank_first[b] = idx
            bank_last[b] = idx

        ht2og = {}
        for og, (ht_lo, ht_hi) in enumerate(STORE_GROUPS):
            for t in range(ht_lo, ht_hi):
                ht2og[t] = og

        def pass2_ht(ht):
            og = ht2og[ht]
            ht_lo, ht_hi = STORE_GROUPS[og]
            a = ht - ht_lo
            ogrp = ht_hi - ht_lo
            if og not in yt_tiles:
                yt_tiles[og] = out_pool.tile(
                    [P, ogrp * w], u8, tag="yt", name=f"yt{og}"
                )
                done_ht[og] = 0
            yt = yt_tiles[og]
            qtile = {}
            for idx, (wt, lo, hi) in enumerate(pieces_w):
                q = lo // HALF
                if q not in qtile:
                    qtile[q] = next_psB()
                base = P * wt - RADIUS
                b = lo // BANK
                nc.tensor.matmul(
                    qtile[q][:, lo - q * HALF : hi - q * HALF],
                    s1_tiles[(ht_band[ht][0], wt)][
                        :, ht * P - ht_band[ht][1] : (ht + 1) * P - ht_band[ht][1]
                    ],
                    band8_t[:, lo - base : hi - base],
                    start=(idx == bank_first[b]),
                    stop=(idx == bank_last[b]),
                )
                if idx == q_last[q]:
                    evac_p2(
                        yt[:, a * w + q * HALF : a * w + (q + 1) * HALF],
                        qtile[q],
                        HALF,
                    )
            done_ht[og] += 1
            if done_ht[og] == ogrp:
                dst = y_ap[ht_lo * P : ht_hi * P, :].rearrange(
                    "(a p) w -> p a w", p=P
                )
                # Early stores ride gpsimd's SWDGE queue, which is
                # backlogged behind the input cast-DMA descriptor gen --
                # this keeps them from stealing HBM bandwidth from the
                # input stream.  The tail-critical last stores go on sync
                # HWDGE for minimum latency.
                eng = nc.sync if og >= len(STORE_GROUPS) - 2 else nc.gpsimd
                eng.dma_start(
                    out=dst, in_=yt[:].rearrange("p (a w) -> p a w", a=ogrp)
                )

        # --- Main loop: pass-1 bands, interleaved with pass-2 -------------
        for bi, (blo, bhi) in enumerate(bands):
            kt_lo = max(0, (blo - RADIUS) // P)
            kt_hi = min(nt_h - 1, (bhi + RADIUS - 1) // P)
            cs = cnt_slice_for(blo, bhi)
            for wt in range(nt_w):
                psA = next_psA()
                for kt in range(kt_lo, kt_hi + 1):
                    win_lo = max(blo, P * kt - RADIUS)
                    win_hi = min(bhi, P * kt + P + RADIUS)
                    base = P * kt - RADIUS
                    nc.tensor.matmul(
                        psA[:, win_lo - blo : win_hi - blo],
                        m_tiles[kt][:, wt * P : (wt + 1) * P],
                        band8_t[:, win_lo - base : win_hi - base],
                        start=(kt == kt_lo),
                        stop=(kt == kt_hi),
                    )
                evac_p1(s1_tiles[(bi, wt)][:, 0 : bhi - blo], psA, cs, blo, bhi)

            for ht in range(blo // P, bhi // P):
                pass2_ht(ht)

    _dedupe_ldweights(nc)
    nc.compile()
    return nc


def kernel(x: np.ndarray) -> np.ndarray:
    """Full-input entry point: x (8, 2048, 2048, 1) f32 -> y same shape uint8."""
    from concourse.bass_utils import run_bass_kernel_spmd

    x = np.asarray(x)
    assert x.shape == (N_CORES, H, W, 1), x.shape
    imgs = np.ascontiguousarray(x[:, :, :, 0], dtype=np.float32)

    nc = build_program(H, W)
    band8 = band_np().astype(ml_dtypes.float8_e4m3)
    cnt = cnt_np().astype(ml_dtypes.bfloat16)
    in_maps = [
        {"x": imgs[c], "band8": band8, "cnt": cnt} for c in range(N_CORES)
    ]
    res = run_bass_kernel_spmd(nc, in_maps, list(range(N_CORES)))
    y = np.stack([res.results[c]["y"] for c in range(N_CORES)])
    return y[..., None]


# revision 52
# speedup vs baseline: 1.0282x; 1.0282x over previous
"""Trainium2 Bass kernel for nn_DilatedMask: 33x33 binary mask dilation.

Computes, for x of shape (8, 2048, 2048, 1) float32 with values {0,1}:
    mask = (x == 0)
    y    = sliding-window max of mask over a 33x33 window (SAME padding),
           as uint8.

Strategy (per NeuronCore, pure data parallel over the batch of 8):
  y = 1 iff the 33x33 window contains a zero of x.  Equivalently, with
  boxsum_h(x) the 33-tap column sum (clipped at borders) and count_h the
  per-row tap count:
      s1b = (boxsum_h(x) < count_h)        "column-window has a zero"
      y   = (boxsum_w(s1b) > 0)            "any such column in w-window"
  Both banded sums run on the TensorEngine as matmuls with the image tile
  as the *stationary* operand, which transposes each pass's output:
  pass 1 emits S1^T [w, h]; pass 2 lands back in natural [h, w].

  The input streams in via gpsimd SWDGE cast-DMAs (f32 -> fp8 during the
  DMA), so no engine ever touches the f32 data: the PE consumes x
  directly and the thresholds absorb the mask inversion.  PSUM
  evacuations (the compare ops) round-robin across DVE / ACT / GpSimd.

Pass-1 is banded over h_out (bands below); within a band, per w-tile all
contributing k-strips accumulate into one PSUM bank.  Pass-2 runs per
output row-strip with the w-tile loop outermost so each stationary loads
exactly once per strip.
"""

from contextlib import ExitStack

import numpy as np
import ml_dtypes

RADIUS = 16
SE = 2 * RADIUS + 1  # 33
P = 128
BANK = 512  # PSUM bank width in f32 elements
H = W = 2048
N_CORES = 8

# Pass-1 h_out bands: narrow at the start so the evac/pass-2 engines get
# work as soon as the first strips land; 512-wide in the middle to keep
# LDWEIGHTS duplication low; 256 at the end so the post-stream tail (work
# gated on the final strip) is short.
BANDS_H = [(0, 128), (128, 256), (256, 512), (512, 1024), (1024, 1536),
           (1536, 1792), (1792, 2048)]


def band_np() -> np.ndarray:
    """Band matrix chunk [128, 160]: band[p, j] = 1 iff j-32 <= p <= j."""
    p = np.arange(P)[:, None]
    j = np.arange(P + 2 * RADIUS)[None, :]
    return ((p <= j) & (p >= j - 2 * RADIUS)).astype(np.float32)


def cnt_np() -> np.ndarray:
    """Threshold counts, bf16 [128, 768]:
    cols 0..511   -> count_h(j) for h_out j in [0, 512)
    cols 512..767 -> count_h(1792 + j) for j in [0, 256)
    count_h(i) = min(i + 16, 2047) - max(i - 16, 0) + 1
    """
    i = np.arange(H)
    cnt = np.minimum(i + RADIUS, H - 1) - np.maximum(i - RADIUS, 0) + 1
    cols = np.concatenate([cnt[0:512], cnt[1792:2048]]).astype(np.float32)
    return np.broadcast_to(cols[None, :], (P, 768)).copy()


def _dedupe_ldweights(nc):
    """Remove back-to-back duplicate LDWEIGHTS in the PE stream.

    Tile lowers every matmul to LDWEIGHTS+MATMUL; consecutive matmuls that
    share a stationary reload identical weights.  The PE pairs each MATMUL
    with the most recent preceding LDWEIGHTS, so the reload is dead.
    Only drops loads with empty sync_info.
    """
    import concourse.mybir as mybir

    for blk in nc.m.functions[0].blocks:
        insts = list(blk.instructions)
        keep = []
        remap = {}
        last_sig = None
        for i in insts:
            if i.engine == mybir.EngineType.PE:
                tn = type(i).__name__
                if tn == "InstLdweights":
                    ap = i.ins[0]
                    sig = (ap.memref, ap.offset, str(ap.ap), str(ap.dtype),
                           bool(i.is_transpose), str(i.perf_mode))
                    si = i.sync_info
                    clean = si is None or (
                        not si.on_wait and not si.on_update
                    )
                    if sig == last_sig and clean:
                        remap[i.name] = last_name
                        continue
                    last_sig = sig
                    last_name = i.name
                elif tn != "InstMatmult":
                    last_sig = None
            keep.append(i)
        if remap:
            for i in keep:
                i.remap_dependency_names(remap)
            blk.instructions = keep


def build_program(h: int = H, w: int = W):
    """Build the per-core Bass program (SPMD, identical on all cores)."""
    import concourse.bass as bass
    import concourse.mybir as mybir
    import concourse.tile as tile
    from concourse import bacc

    f32 = mybir.dt.float32
    fp8 = mybir.dt.float8e4
    bf16 = mybir.dt.bfloat16
    u8 = mybir.dt.uint8

    nt_h = h // P
    nt_w = w // P
    bands = BANDS_H

    nc = bacc.Bacc("TRN2", target_bir_lowering=False, debug=False)
    # const AP for the ACT sign(33 - x) bias
    _t33 = nc.alloc_sbuf_tensor("const-f32-33", [P, 1], f32)
    nc.gpsimd.memset(_t33.ap(), 33.0)
    nc.const_aps.aps[(f32, 33.0)] = _t33.ap()
    x_ap = nc.dram_tensor("x", [h, w], f32, kind="ExternalInput").ap()
    band8_ap = nc.dram_tensor(
        "band8", [P, P + 2 * RADIUS], fp8, kind="ExternalInput"
    ).ap()
    cnt_ap = nc.dram_tensor("cnt", [P, 768], bf16, kind="ExternalInput").ap()
    y_ap = nc.dram_tensor("y", [h, w], u8, kind="ExternalOutput").ap()

    # Output row-strip groups per store DMA: pairs through the body, single
    # strips at the end so the final (fully exposed) store is small and the
    # second-to-last overlaps the last strip's compute.
    STORE_GROUPS = [(0, 2), (2, 4), (4, 6), (6, 8), (8, 10), (10, 12),
                    (12, 14), (14, 16)]

    with tile.TileContext(nc) as tc, ExitStack() as ctx:
        const_pool = ctx.enter_context(tc.tile_pool(name="const", bufs=1))
        m_pool = ctx.enter_context(tc.tile_pool(name="m", bufs=nt_h))
        s1_pool = ctx.enter_context(tc.tile_pool(name="s1", bufs=nt_w))
        psA_pool = ctx.enter_context(tc.tile_pool(name="psA", bufs=4, space="PSUM"))
        psB_pool = ctx.enter_context(tc.tile_pool(name="psB", bufs=2, space="PSUM"))
        # One buffer per store group: a shared rotation would make late
        # evacs wait on the (deliberately delayed) early stores.
        out_pool = ctx.enter_context(
            tc.tile_pool(name="out", bufs=len(STORE_GROUPS))
        )

        # Input: 16 row strips, cast f32 -> fp8 {0,1} inside the SWDGE DMA.
        # All gens are emitted first so the gpsimd sequencer finishes
        # descriptor generation before any evac work lands on it.
        # The last three strips load in two w-halves each: the final band's
        # matmuls gate the whole post-stream tail, and a half-strip lets
        # their low-w pieces start ~1.2us earlier.
        m_tiles = []
        for kt in range(nt_h):
            m = m_pool.tile([P, w], fp8, tag="m", name=f"m{kt}")
            if kt >= nt_h - 3:
                hw_ = w // 2
                nc.gpsimd.dma_start(
                    out=m[:, 0:hw_],
                    in_=x_ap[kt * P : (kt + 1) * P, 0:hw_],
                )
                nc.gpsimd.dma_start(
                    out=m[:, hw_:w],
                    in_=x_ap[kt * P : (kt + 1) * P, hw_:w],
                )
            else:
                nc.gpsimd.dma_start(
                    out=m[:], in_=x_ap[kt * P : (kt + 1) * P, :]
                )
            m_tiles.append(m)

        band8_t = const_pool.tile([P, P + 2 * RADIUS], fp8, tag="band8")
        nc.sync.dma_start(out=band8_t[:], in_=band8_ap[:, :])
        cnt_t = const_pool.tile([P, 768], bf16, tag="cnt")
        nc.sync.dma_start(out=cnt_t[:], in_=cnt_ap[:, :])

        # Persistent PSUM tiles (same memref -> PE WAW stays program-order).
        psA_tiles = [
            psA_pool.tile([P, BANK], f32, tag="psA", name=f"psA{i}")
            for i in range(4)
        ]
        psB_tiles = [
            psB_pool.tile([P, 2 * BANK], f32, tag="psB", name=f"psB{i}")
            for i in range(2)
        ]
        nA = [0]
        nB = [0]

        def next_psA():
            t = psA_tiles[nA[0] % len(psA_tiles)]
            nA[0] += 1
            return t

        def next_psB():
            t = psB_tiles[nB[0] % len(psB_tiles)]
            nB[0] += 1
            return t

        # s1 strips: S1^T[wt] [w', h] in fp8.
        s1_tiles = [
            s1_pool.tile([P, h], fp8, tag="s1", name=f"s1_{wt}")
            for wt in range(nt_w)
        ]
        yt_tiles = {}
        done_ht = {}

        # --- Evacuation engines -------------------------------------------
        # PSUM -> SBUF compares split across DVE and ACT (gpsimd has no
        # PSUM access on TRN2).  Greedy balance on estimated cost; compares
        # against the cnt tile need tensor_tensor and are DVE-only.
        ev_cost = {"v": 0.0, "a": 0.0}

        # Fixed costs include the ~2-3 Tile semaphore ops each evac puts on
        # the engine queue (~0.18us each), which showed up as real ACT/DVE
        # occupancy in traces.
        def pick_engine(cols, force_v=False):
            cv = ev_cost["v"] + 0.13 + cols * 1.07e-3
            ca = ev_cost["a"] + 0.25 + cols * 0.95e-3
            eng = "v" if (force_v or cv <= ca) else "a"
            ev_cost[eng] = cv if eng == "v" else ca
            return eng

        def evac_p1_uniform(dst_ap, src_ap, cols):
            eng = pick_engine(cols)
            if eng == "a":
                # sign(33 - boxsum): 1 iff boxsum < 33
                nc.scalar.activation(
                    dst_ap, src_ap, mybir.ActivationFunctionType.Sign,
                    bias=33.0, scale=-1.0,
                )
            else:
                nc.vector.tensor_scalar(dst_ap, src_ap, 33.0, None,
                                        mybir.AluOpType.is_lt)

        def evac_p1(dst_ap, psA, cnt_slice, blo, bhi):
            """s1b = (boxsum < count).  cnt_slice None -> uniform 33.

            Only the outermost RADIUS h_out columns have counts != 33; the
            cnt compare (DVE-only tensor_tensor) covers just those 16
            columns so the bulk stays eligible for either engine.
            """
            cols = bhi - blo
            if cnt_slice is None:
                evac_p1_uniform(dst_ap, psA[:, 0:cols], cols)
                return
            if blo < RADIUS:  # first band: edge cols at the front
                ev_cost["v"] += 0.16
                nc.vector.tensor_tensor(
                    dst_ap[:, 0:RADIUS], psA[:, 0:RADIUS],
                    cnt_slice[:, 0:RADIUS], mybir.AluOpType.is_lt,
                )
                evac_p1_uniform(
                    dst_ap[:, RADIUS:cols], psA[:, RADIUS:cols], cols - RADIUS
                )
            else:  # last band: edge cols at the back
                e = cols - RADIUS
                ev_cost["v"] += 0.16
                nc.vector.tensor_tensor(
                    dst_ap[:, e:cols], psA[:, e:cols],
                    cnt_slice[:, e:cols], mybir.AluOpType.is_lt,
                )
                evac_p1_uniform(dst_ap[:, 0:e], psA[:, 0:e], e)

        def evac_p2(dst_ap, psB, cols):
            """y = (boxsum_w > 0)."""
            eng = pick_engine(cols)
            src_ap = psB[:, 0:cols]
            if eng == "a":
                nc.scalar.sign(dst_ap, src_ap)
            else:
                nc.vector.tensor_scalar(dst_ap, src_ap, 0.5, None,
                                        mybir.AluOpType.is_gt)

        def cnt_slice_for(blo, bhi):
            """cnt tile slice for band [blo,bhi) or None if uniformly 33."""
            if blo < RADIUS:  # first band
                return cnt_t[:, blo : bhi]
            if bhi > h - RADIUS:  # last band
                off = 512 + (blo - 1792)
                return cnt_t[:, off : off + (bhi - blo)]
            return None

        # --- Pass 2 for one output row-strip ------------------------------
        # Matmul pieces must stay within one PSUM bank (512 f32); the
        # evac granularity is a psB half [128, 1024] (2 banks).
        HALF = 2 * BANK
        pieces_w = []  # (wt, lo, hi) over full w, split at BANK boundaries
        for wt in range(nt_w):
            win_lo = max(0, P * wt - RADIUS)
            win_hi = min(w, P * wt + P + RADIUS)
            lo = win_lo
            while lo < win_hi:
                hi = min(win_hi, (lo // BANK + 1) * BANK)
                pieces_w.append((wt, lo, hi))
                lo = hi
        q_last = {}
        bank_first = {}
        bank_last = {}
        for idx, (wt, lo, hi) in enumerate(pieces_w):
            q_last[lo // HALF] = idx
            b = lo // BANK
            if b not in bank_first:
                bank_first[b] = idx
            bank_last[b] = idx

        ht2og = {}
        for og, (ht_lo, ht_hi) in enumerate(STORE_GROUPS):
            for t in range(ht_lo, ht_hi):
                ht2og[t] = og

        def pass2_ht(ht):
            og = ht2og[ht]
            ht_lo, ht_hi = STORE_GROUPS[og]
            a = ht - ht_lo
            ogrp = ht_hi - ht_lo
            if og not in yt_tiles:
                yt_tiles[og] = out_pool.tile(
                    [P, ogrp * w], u8, tag="yt", name=f"yt{og}"
                )
                done_ht[og] = 0
            yt = yt_tiles[og]
            qtile = {}
            for idx, (wt, lo, hi) in enumerate(pieces_w):
                q = lo // HALF
                if q not in qtile:
                    qtile[q] = next_psB()
                base = P * wt - RADIUS
                b = lo // BANK
                nc.tensor.matmul(
                    qtile[q][:, lo - q * HALF : hi - q * HALF],
                    s1_tiles[wt][:, ht * P : (ht + 1) * P],
                    band8_t[:, lo - base : hi - base],
                    start=(idx == bank_first[b]),
                    stop=(idx == bank_last[b]),
                )
                if idx == q_last[q]:
                    evac_p2(
                        yt[:, a * w + q * HALF : a * w + (q + 1) * HALF],
                        qtile[q],
                        HALF,
                    )
            done_ht[og] += 1
            if done_ht[og] == ogrp:
                dst = y_ap[ht_lo * P : ht_hi * P, :].rearrange(
                    "(a p) w -> p a w", p=P
                )
                # Early stores ride gpsimd's SWDGE queue, which is
                # backlogged behind the input cast-DMA descriptor gen --
                # this keeps them from stealing HBM bandwidth from the
                # input stream.  The tail-critical last stores go on sync
                # HWDGE for minimum latency.
                eng = nc.sync if og >= len(STORE_GROUPS) - 2 else nc.gpsimd
                eng.dma_start(
                    out=dst, in_=yt[:].rearrange("p (a w) -> p a w", a=ogrp)
                )

        # --- Main loop: pass-1 bands, interleaved with pass-2 -------------
        for bi, (blo, bhi) in enumerate(bands):
            kt_lo = max(0, (blo - RADIUS) // P)
            kt_hi = min(nt_h - 1, (bhi + RADIUS - 1) // P)
            cs = cnt_slice_for(blo, bhi)
            for wt in range(nt_w):
                psA = next_psA()
                for kt in range(kt_lo, kt_hi + 1):
                    win_lo = max(blo, P * kt - RADIUS)
                    win_hi = min(bhi, P * kt + P + RADIUS)
                    base = P * kt - RADIUS
                    nc.tensor.matmul(
                        psA[:, win_lo - blo : win_hi - blo],
                        m_tiles[kt][:, wt * P : (wt + 1) * P],
                        band8_t[:, win_lo - base : win_hi - base],
                        start=(kt == kt_lo),
                        stop=(kt == kt_hi),
                    )
                evac_p1(s1_tiles[wt][:, blo:bhi], psA, cs, blo, bhi)

            for ht in range(blo // P, bhi // P):
                pass2_ht(ht)

    _dedupe_ldweights(nc)
    nc.compile()
    return nc


def kernel(x: np.ndarray) -> np.ndarray:
    """Full-input entry point: x (8, 2048, 2048, 1) f32 -> y same shape uint8."""
    from concourse.bass_utils import run_bass_kernel_spmd

    x = np.asarray(x)
    assert x.shape == (N_CORES, H, W, 1), x.shape
    imgs = np.ascontiguousarray(x[:, :, :, 0], dtype=np.float32)

    nc = build_program(H, W)
    band8 = band_np().astype(ml_dtypes.float8_e4m3)
    cnt = cnt_np().astype(ml_dtypes.bfloat16)
    in_maps = [
        {"x": imgs[c], "band8": band8, "cnt": cnt} for c in range(N_CORES)
    ]
    res = run_bass_kernel_spmd(nc, in_maps, list(range(N_CORES)))
    y = np.stack([res.results[c]["y"] for c in range(N_CORES)])
    return y[..., None]
